# revision 7
# baseline (speedup 1.0000x reference)
"""Trainium2 Bass kernel for nn_DCAM (dense transformer attention module).

Reference computation (per batch b):
  qp/kp/vp = avg_pool2d(feature_{q,k,v}, 2)            # (C=256, 64, 64)
  q = Wq @ qp, k = Wk @ kp  (M=32 channels)            # (32, N=4096)
  v = Wv @ vp                                          # (256, N)
  attn = softmax(q^T k, axis=-1)                       # (N, N)
  out[c, m] = sum_n v[c, n] attn[m, n]                 # (256, N)
  result = upsample_nearest(out, 2) + feature_v        # (256, 128, 128)

Sharding: data-parallel over batch B=8 across 8 NeuronCores (1 batch/core).

Per-core design (v2 - single-precision bf16, pooling folded into PE):
  - Inputs stream HBM->SBUF as f32->bf16 cast DMAs (SWDGE). 2x2 sum-pooling
    is never computed by DVE/GPSIMD: for q/k it's folded into the projection
    matmuls as 4 PSUM-accumulated terms with stride-2 access patterns; for v,
    vertical pairs are pre-summed by one DVE pass and the horizontal fold
    happens in the vT projection (2 strided lhsT terms per cb).
  - Single bf16 term for S and all projections (measured end-to-end rel err
    ~3e-3 vs the 2e-2 gate; hi/lo splits buy nothing).
  - S^T computed directly (lhsT = k j-block, rhs = q i-chunk); K=32 so 4
    j-blocks run concurrently via tile_position row tiling.
  - softmax denominator: bf16 pairwise tree over the 16 P tiles per i-chunk
    on DVE (2 adds on GPSIMD), merged by a 2-matmul ones reduction into PSUM.
  - exp on ACT from PSUM at FD=1024; P written bf16 straight to SBUF.
  - Load order: fq chunk0, then fk/fv chunk pairs with i-chunk 0's attention
    interleaved so compute chases the DMA stream; fq 1..7 trickle during the
    remaining i-chunks.
  - pooling is a 2x2 *sum*; q,k scales fold into the exp scale (1/16), v's
    into WvT (x0.25) on the host.
"""
import numpy as np
import ml_dtypes

import concourse.bass as bass
import concourse.mybir as mybir
import concourse.tile as tile
from concourse import bacc
from concourse.bass_utils import run_bass_kernel_spmd

F32 = mybir.dt.float32
BF16 = mybir.dt.bfloat16
AF = mybir.ActivationFunctionType

B = 8
C = 256
M = 32
H = W = 128
HP = WP = 64
N = HP * WP          # 4096
CB = C // 128        # 2 channel blocks
JB = N // 128        # 32 key blocks
JG = JB // 4         # 8 groups of 4 packed j-blocks
IC = N // 512        # 8 query chunks


def build_module():
    nc = bacc.Bacc("TRN2", target_bir_lowering=False, debug=False)

    fq_d = nc.dram_tensor("feature_q", [C, H, W], F32, kind="ExternalInput").ap()
    fk_d = nc.dram_tensor("feature_k", [C, H, W], F32, kind="ExternalInput").ap()
    fv_d = nc.dram_tensor("feature_v", [C, H, W], F32, kind="ExternalInput").ap()
    wqt_d = nc.dram_tensor("WqT", [C, M], BF16, kind="ExternalInput").ap()
    wkt_d = nc.dram_tensor("WkT", [C, M], BF16, kind="ExternalInput").ap()
    wvt_d = nc.dram_tensor("WvT", [C, C], BF16, kind="ExternalInput").ap()
    out_d = nc.dram_tensor("out", [C, H, W], F32, kind="ExternalOutput").ap()

    with tile.TileContext(nc) as tc:
        with tc.tile_pool(name="const", bufs=1) as cpool, \
             tc.tile_pool(name="persist", bufs=1) as pp, \
             tc.tile_pool(name="ps", bufs=1, space="PSUM") as ps, \
             tc.tile_pool(name="work", bufs=1) as pa, \
             tc.tile_pool(name="dramb", bufs=2, space="DRAM") as dpool:
            # ---- constants ----
            wq_sb = cpool.tile([128, CB, M], BF16)
            nc.sync.dma_start(wq_sb[:], wqt_d.rearrange("(b p) m -> p b m", p=128))
            wk_sb = cpool.tile([128, CB, M], BF16)
            nc.sync.dma_start(wk_sb[:], wkt_d.rearrange("(b p) m -> p b m", p=128))
            wv_sb = cpool.tile([128, CB, C], BF16)
            nc.sync.dma_start(wv_sb[:], wvt_d.rearrange("(b p) c -> p b c", p=128))
            ones_b = cpool.tile([128, 1], BF16)
            nc.vector.memset(ones_b[:], 1.0)
            # ACT exp table warm-up (the table load costs ~2.7us; do it now,
            # long before the first real exp).
            dum = cpool.tile([1, 8], F32)
            nc.vector.memset(dum[:], 0.0)
            nc.scalar.activation(dum[:], dum[:], AF.Exp, scale=0.0625)

            # ---- persistent tensors ----
            fv_sb = pp.tile([128, CB, H, W], BF16)    # raw fv (residual + vproj)
            vt_all = pp.tile([128, JB, C], BF16)      # vT[j, c] per j-block
            q4h = pp.tile([128, N], BF16)             # q replicated x4 groups
            kh_all = pp.tile([128, JG, 128], BF16)    # [32*(jb%4)+m, jg, jf]

            # ================= helpers =================
            def q_chunk(icn):
                cq = pa.tile([128, CB, 16, W], BF16, tag="cq", bufs=1,
                             name="cq")
                nc.gpsimd.dma_start(
                    cq[:],
                    fq_d[:, icn * 16:(icn + 1) * 16, :].rearrange(
                        "(b p) h w -> p b h w", p=128))
                pr = ps.tile([128, 1024], F32, tag="s", bufs=3,
                             name="pr_q")[:M, :512]
                mm = 0
                for cb in range(CB):
                    cr = cq[:, cb].rearrange("c (h dy) (w dx) -> c h dy w dx",
                                             dy=2, dx=2)
                    for dy in range(2):
                        for dx in range(2):
                            nc.tensor.matmul(pr, wq_sb[:, cb],
                                             cr[:, :, dy, :, dx],
                                             start=(mm == 0), stop=(mm == 7),
                                             skip_group_check=True)
                            mm += 1
                cs = slice(icn * 512, (icn + 1) * 512)
                nc.scalar.copy(q4h[0:32, cs], pr)
                for g in range(1, 4):
                    nc.sync.dma_start(q4h[g * 32:(g + 1) * 32, cs],
                                      q4h[0:32, cs])

            def k_chunk(icn):
                ck = pa.tile([128, CB, 16, W], BF16, tag="ck", bufs=2,
                             name="ck")
                nc.gpsimd.dma_start(
                    ck[:],
                    fk_d[:, icn * 16:(icn + 1) * 16, :].rearrange(
                        "(b p) h w -> p b h w", p=128))
                pr = ps.tile([128, 1024], F32, tag="s", bufs=3,
                             name="pr_k")[:M, :512]
                mm = 0
                for cb in range(CB):
                    cr = ck[:, cb].rearrange("c (h dy) (w dx) -> c h dy w dx",
                                             dy=2, dx=2)
                    for dy in range(2):
                        for dx in range(2):
                            nc.tensor.matmul(pr, wk_sb[:, cb],
                                             cr[:, :, dy, :, dx],
                                             start=(mm == 0), stop=(mm == 7),
                                             skip_group_check=True)
                            mm += 1
                for t in range(4):
                    nc.scalar.copy(kh_all[t * 32:(t + 1) * 32, icn, :],
                                   pr[:, t * 128:(t + 1) * 128])

            def v_chunk(icn):
                # load raw chunk into the persistent residual copy
                nc.gpsimd.dma_start(
                    fv_sb[:, :, icn * 16:(icn + 1) * 16, :],
                    fv_d[:, icn * 16:(icn + 1) * 16, :].rearrange(
                        "(b p) h w -> p b h w", p=128))
                # vertical 2x2 pair sums (one DVE pass, bf16 2x mode)
                vv = pa.tile([128, CB, 8, W], BF16, tag="vv", bufs=2,
                             name="vv")
                for cb in range(CB):
                    fvc = fv_sb[:, cb, icn * 16:(icn + 1) * 16, :].rearrange(
                        "c (h dy) w -> c h dy w", dy=2)
                    nc.vector.tensor_add(vv[:, cb], fvc[:, :, 0], fvc[:, :, 1])
                # vT projection per j-block; horizontal fold via 2 strided
                # lhsT terms per cb.
                for r in range(4):
                    jb = icn * 4 + r
                    vt_ps = ps.tile([128, 1024], F32, tag="s", bufs=3,
                                    name="vt_ps")[:, :C]
                    mm = 0
                    for cb in range(CB):
                        vr = vv[:, cb, 2 * r:2 * r + 2, :].rearrange(
                            "c h (w dx) -> c h w dx", dx=2)
                        for dx in range(2):
                            nc.tensor.matmul(vt_ps, vr[:, :, :, dx],
                                             wv_sb[:, cb],
                                             start=(mm == 0), stop=(mm == 3),
                                             skip_group_check=True)
                            mm += 1
                    nc.scalar.copy(vt_all[:, jb, :], vt_ps)

            def b_s_exp(ic, jg, t1s, t2s):
                i0 = ic * 512
                # allocate both S tiles, then issue all 4 matmuls adjacent so
                # the four 32-row tile_position groups run concurrently.
                s_t = [ps.tile([128, 1024], F32, tag="s", bufs=3, name="s_u")
                       for _ in range(2)]
                for t in range(4):
                    gs = slice(t * 32, (t + 1) * 32)
                    nc.tensor.matmul(
                        s_t[t // 2][:, (t % 2) * 512:(t % 2) * 512 + 512],
                        kh_all[gs, jg, :], q4h[gs, i0:i0 + 512],
                        start=True, stop=True,
                        tile_position=(t * 32, 0),
                        skip_group_check=True)
                p_t = []
                for u in range(2):
                    p = pa.tile([128, 1024], BF16, tag="p", bufs=16, name="p")
                    nc.scalar.activation(p[:], s_t[u][:], AF.Exp, scale=0.0625)
                    p_t.append(p)
                # denominator tree level 1 (jg 1,3 -> GPSIMD for balance),
                # level 2 inlined on odd jg, first half of level 3 after jg 3.
                t1 = pa.tile([128, 1024], BF16, tag="t1", bufs=3, name="t1")
                if ic > 0 and jg in (1, 3):
                    nc.gpsimd.tensor_add(t1[:], p_t[0][:], p_t[1][:])
                else:
                    nc.vector.tensor_add(t1[:], p_t[0][:], p_t[1][:])
                t1s.append(t1)
                if jg % 2 == 1:
                    t2 = pa.tile([128, 1024], BF16, tag="t2", bufs=4,
                                 name="t2")
                    nc.vector.tensor_add(t2[:], t1s[-2][:], t1s[-1][:])
                    t2s.append(t2)
                if jg == 3:
                    t3 = pa.tile([128, 1024], BF16, tag="t3", bufs=2,
                                 name="t3")
                    nc.vector.tensor_add(t3[:], t2s[0][:], t2s[1][:])
                    t2s.append(t3)  # t2s = [t2_0, t2_1, t3a, t2_2, t2_3...]
                return p_t

            def b_pv(jg, o_ps, p_t):
                for u in range(2):
                    for tt in range(2):
                        j = jg * 4 + u * 2 + tt
                        pr = p_t[u][:, tt * 512:tt * 512 + 512]
                        for cb in range(CB):
                            nc.tensor.matmul(
                                o_ps[cb],
                                vt_all[:, j, cb * 128:(cb + 1) * 128],
                                pr,
                                start=(j == 0), stop=(j == JB - 1),
                                skip_group_check=True)

            def b_tail1(ic, o_ps, t1s, t2s):
                # drain the PV accumulators to SBUF first so the next
                # i-chunk's first PV matmul isn't blocked on the epilogue.
                od = pa.tile([128, CB, 512], BF16, tag="od", bufs=2, name="od")
                for cb in range(CB):
                    nc.vector.tensor_copy(od[:, cb], o_ps[cb][:])
                # tree remainder: t2s holds [t2_0, t2_1, t3a, t2_2, t2_3]
                t3b = pa.tile([128, 1024], BF16, tag="t3", bufs=2, name="t3b")
                nc.vector.tensor_add(t3b[:], t2s[3][:], t2s[4][:])
                tT = pa.tile([128, 1024], BF16, tag="tT", bufs=2, name="tT")
                nc.vector.tensor_add(tT[:], t2s[2][:], t3b[:])
                # l = column sums of both halves via 2 ones-matmuls
                l_ps = ps.tile([128, 1024], F32, tag="s", bufs=3,
                               name="l_ps")[:1, :512]
                for u in range(2):
                    nc.tensor.matmul(l_ps, ones_b[:],
                                     tT[:, u * 512:(u + 1) * 512],
                                     start=(u == 0), stop=(u == 1),
                                     skip_group_check=True)
                l_sb = pa.tile([1, 512], F32, tag="l_sb", bufs=2, name="l_sb")
                nc.scalar.copy(l_sb[:], l_ps)
                # transpose to (128, 4) via DRAM bounce, reciprocal, back
                l_dr = dpool.tile([512], F32, tag="l_dr", bufs=2, name="l_dr")
                nc.sync.dma_start(l_dr[:], l_sb[:])
                lT = pa.tile([128, 4], F32, tag="lT", bufs=2, name="lT")
                nc.sync.dma_start(lT[:], l_dr.rearrange("(p b) -> p b", b=4))
                rT = pa.tile([128, 4], F32, tag="rT", bufs=2, name="rT")
                nc.vector.reciprocal(rT[:], lT[:])
                r_dr = dpool.tile([512], F32, tag="r_dr", bufs=2, name="r_dr")
                nc.sync.dma_start(r_dr.rearrange("(p b) -> p b", b=4), rT[:])
                rb_sb = pa.tile([128, 512], F32, tag="rb_sb", bufs=2,
                                name="rb_sb")
                nc.sync.dma_start(
                    rb_sb[:],
                    r_dr.rearrange("(o x) -> o x", o=1).to_broadcast(
                        (128, 512)))
                return od, rb_sb

            def b_tail2(ic, od, rb_sb):
                for cb in range(CB):
                    oc = pa.tile([128, 512], F32, tag="oc", bufs=2, name="oc")
                    nc.vector.tensor_mul(oc[:], od[:, cb], rb_sb[:])
                    for sl in range(2):
                        final = pa.tile([128, 4, 2, WP, 2], F32, tag="final",
                                        bufs=3, name="final")
                        up = oc.rearrange("c (h w) -> c h w", w=WP)[
                            :, sl * 4:(sl + 1) * 4, :, None].to_broadcast(
                                (128, 4, WP, 2))
                        r0 = ic * 16 + sl * 8
                        fvv = fv_sb[:, cb, r0:r0 + 8, :].rearrange(
                            "c (h dy) (w dx) -> c h dy w dx", dy=2, dx=2)
                        nc.vector.tensor_add(final[:, :, 0], up, fvv[:, :, 0])
                        nc.gpsimd.tensor_add(final[:, :, 1], up,
                                             fvv[:, :, 1])
                        nc.sync.dma_start(
                            out_d[cb * 128:(cb + 1) * 128, r0:r0 + 8, :],
                            final.rearrange("c h dy w dx -> c (h dy) (w dx)"))

            # ================= schedule =================
            # i-chunk 0's S+exp chases the fk stream and its PV chases the
            # fv stream; i-chunks 1..7 run as one flat software-pipelined
            # (ic, jg) loop with the S+exp stage issued one unit ahead so
            # the exp for unit n+1 overlaps the PV matmuls of unit n and
            # the PE never idles into a HAM re-throttle.
            st = {}

            def new_state(ic):
                st[ic] = dict(
                    o=[ps.tile([128, 512], F32, tag=f"o{cb}", bufs=1,
                               name=f"o{cb}_ps") for cb in range(CB)],
                    t1s=[], t2s=[], p={})

            def s_exp_unit(ic, jg):
                if jg == 0:
                    new_state(ic)
                s = st[ic]
                s["p"][jg] = b_s_exp(ic, jg, s["t1s"], s["t2s"])

            q_chunk(0)
            for icn in range(IC):
                k_chunk(icn)
                s_exp_unit(0, icn)
            q_chunk(1)
            for icn in range(IC):
                v_chunk(icn)
                if icn == IC - 1:
                    s_exp_unit(1, 0)
                b_pv(icn, st[0]["o"], st[0]["p"][icn])
            pend = b_tail1(0, st[0]["o"], st[0]["t1s"], st[0]["t2s"])
            pend_ic = 0

            units = [(ic, jg) for ic in range(1, IC) for jg in range(JG)]
            for idx, (ic, jg) in enumerate(units):
                if jg == 0 and ic + 1 < IC:
                    q_chunk(ic + 1)
                if idx + 1 < len(units):
                    s_exp_unit(*units[idx + 1])
                b_pv(jg, st[ic]["o"], st[ic]["p"].pop(jg))
                if jg == 4 and pend is not None:
                    b_tail2(pend_ic, *pend)
                    pend = None
                if jg == JG - 1:
                    pend = b_tail1(ic, st[ic]["o"], st[ic]["t1s"],
                                   st[ic]["t2s"])
                    pend_ic = ic
                    del st[ic]
            b_tail2(pend_ic, *pend)

    nc.compile()
    return nc


_NC_CACHE = []
LAST_RESULT = []  # last BassKernelResults, for perf inspection by test.py


def kernel(**inputs) -> np.ndarray:
    fq = np.ascontiguousarray(np.asarray(inputs["feature_q"], dtype=np.float32))
    fk = np.ascontiguousarray(np.asarray(inputs["feature_k"], dtype=np.float32))
    fv = np.ascontiguousarray(np.asarray(inputs["feature_v"], dtype=np.float32))
    wq = np.asarray(inputs["Wq"], dtype=np.float32)
    wk = np.asarray(inputs["Wk"], dtype=np.float32)
    wv = np.asarray(inputs["Wv"], dtype=np.float32)

    # weight layout prep (pure layout/scale folding, no heavy compute):
    # on-device pooling is a 2x2 *sum*; q,k each pick up 4x -> s is 16x,
    # folded into the on-device exp scale; v's 4x is folded into WvT here.
    wqt = np.ascontiguousarray(wq.T.astype(ml_dtypes.bfloat16))
    wkt = np.ascontiguousarray(wk.T.astype(ml_dtypes.bfloat16))
    wvt = np.ascontiguousarray(
        (wv.T * 0.25).astype(ml_dtypes.bfloat16))     # (C, C) [c_in, c_out]

    if not _NC_CACHE:
        _NC_CACHE.append(build_module())
    nc = _NC_CACHE[0]

    in_maps = [
        {
            "feature_q": fq[b],
            "feature_k": fk[b],
            "feature_v": fv[b],
            "WqT": wqt,
            "WkT": wkt,
            "WvT": wvt,
        }
        for b in range(B)
    ]
    res = run_bass_kernel_spmd(nc, in_maps, core_ids=list(range(B)))
    LAST_RESULT.clear()
    LAST_RESULT.append(res)
    out = np.stack([res.results[b]["out"] for b in range(B)], axis=0)
    return out.astype(np.float32)


if __name__ == "__main__":
    nc = build_module()
    print("module built + compiled OK")


# revision 9
# speedup vs baseline: 1.0185x; 1.0185x over previous
"""Trainium2 Bass kernel for nn_DCAM (dense transformer attention module).

Reference computation (per batch b):
  qp/kp/vp = avg_pool2d(feature_{q,k,v}, 2)            # (C=256, 64, 64)
  q = Wq @ qp, k = Wk @ kp  (M=32 channels)            # (32, N=4096)
  v = Wv @ vp                                          # (256, N)
  attn = softmax(q^T k, axis=-1)                       # (N, N)
  out[c, m] = sum_n v[c, n] attn[m, n]                 # (256, N)
  result = upsample_nearest(out, 2) + feature_v        # (256, 128, 128)

Sharding: data-parallel over batch B=8 across 8 NeuronCores (1 batch/core).

Per-core design (v2 - single-precision bf16, pooling folded into PE):
  - Inputs stream HBM->SBUF as f32->bf16 cast DMAs (SWDGE). 2x2 sum-pooling
    is never computed by DVE/GPSIMD: for q/k it's folded into the projection
    matmuls as 4 PSUM-accumulated terms with stride-2 access patterns; for v,
    vertical pairs are pre-summed by one DVE pass and the horizontal fold
    happens in the vT projection (2 strided lhsT terms per cb).
  - Single bf16 term for S and all projections (measured end-to-end rel err
    ~3e-3 vs the 2e-2 gate; hi/lo splits buy nothing).
  - S^T computed directly (lhsT = k j-block, rhs = q i-chunk); K=32 so 4
    j-blocks run concurrently via tile_position row tiling.
  - softmax denominator: bf16 pairwise tree over the 16 P tiles per i-chunk
    on DVE (2 adds on GPSIMD), merged by a 2-matmul ones reduction into PSUM.
  - exp on ACT from PSUM at FD=1024; P written bf16 straight to SBUF.
  - Load order: fq chunk0, then fk/fv chunk pairs with i-chunk 0's attention
    interleaved so compute chases the DMA stream; fq 1..7 trickle during the
    remaining i-chunks.
  - pooling is a 2x2 *sum*; q,k scales fold into the exp scale (1/16), v's
    into WvT (x0.25) on the host.
"""
import numpy as np
import ml_dtypes

import concourse.bass as bass
import concourse.mybir as mybir
import concourse.tile as tile
from concourse import bacc
from concourse.bass_utils import run_bass_kernel_spmd

F32 = mybir.dt.float32
BF16 = mybir.dt.bfloat16
AF = mybir.ActivationFunctionType

B = 8
C = 256
M = 32
H = W = 128
HP = WP = 64
N = HP * WP          # 4096
CB = C // 128        # 2 channel blocks
JB = N // 128        # 32 key blocks
JG = JB // 4         # 8 groups of 4 packed j-blocks
IC = N // 512        # 8 query chunks


def build_module():
    nc = bacc.Bacc("TRN2", target_bir_lowering=False, debug=False)

    fq_d = nc.dram_tensor("feature_q", [C, H, W], F32, kind="ExternalInput").ap()
    fk_d = nc.dram_tensor("feature_k", [C, H, W], F32, kind="ExternalInput").ap()
    fv_d = nc.dram_tensor("feature_v", [C, H, W], F32, kind="ExternalInput").ap()
    wqt_d = nc.dram_tensor("WqT", [C, M], BF16, kind="ExternalInput").ap()
    wkt_d = nc.dram_tensor("WkT", [C, M], BF16, kind="ExternalInput").ap()
    wvt_d = nc.dram_tensor("WvT", [C, C], BF16, kind="ExternalInput").ap()
    out_d = nc.dram_tensor("out", [C, H, W], F32, kind="ExternalOutput").ap()

    with tile.TileContext(nc) as tc:
        with tc.tile_pool(name="const", bufs=1) as cpool, \
             tc.tile_pool(name="persist", bufs=1) as pp, \
             tc.tile_pool(name="ps", bufs=1, space="PSUM") as ps, \
             tc.tile_pool(name="work", bufs=1) as pa, \
             tc.tile_pool(name="dramb", bufs=2, space="DRAM") as dpool:
            # ---- constants ----
            wq_sb = cpool.tile([128, CB, M], BF16)
            nc.sync.dma_start(wq_sb[:], wqt_d.rearrange("(b p) m -> p b m", p=128))
            wk_sb = cpool.tile([128, CB, M], BF16)
            nc.sync.dma_start(wk_sb[:], wkt_d.rearrange("(b p) m -> p b m", p=128))
            wv_sb = cpool.tile([128, CB, C], BF16)
            nc.sync.dma_start(wv_sb[:], wvt_d.rearrange("(b p) c -> p b c", p=128))
            ones_b = cpool.tile([128, 1], BF16)
            nc.vector.memset(ones_b[:], 1.0)
            # ACT exp table warm-up (the table load costs ~2.7us; do it now,
            # long before the first real exp).
            dum = cpool.tile([1, 8], F32)
            nc.vector.memset(dum[:], 0.0)
            nc.scalar.activation(dum[:], dum[:], AF.Exp, scale=0.0625)

            # ---- persistent tensors ----
            fv_sb = pp.tile([128, CB, H, W], BF16)    # raw fv (residual + vproj)
            vt_all = pp.tile([128, JB, C], BF16)      # vT[j, c] per j-block
            q4h = pp.tile([128, N], BF16)             # q replicated x4 groups
            kh_all = pp.tile([128, JG, 128], BF16)    # [32*(jb%4)+m, jg, jf]

            # ================= helpers =================
            def q_chunk(icn):
                cq = pa.tile([128, CB, 16, W], BF16, tag="cq", bufs=1,
                             name="cq")
                nc.gpsimd.dma_start(
                    cq[:],
                    fq_d[:, icn * 16:(icn + 1) * 16, :].rearrange(
                        "(b p) h w -> p b h w", p=128))
                pr = ps.tile([128, 1024], F32, tag="s", bufs=2,
                             name="pr_q")[:M, :512]
                mm = 0
                for cb in range(CB):
                    cr = cq[:, cb].rearrange("c (h dy) (w dx) -> c h dy w dx",
                                             dy=2, dx=2)
                    for dy in range(2):
                        for dx in range(2):
                            nc.tensor.matmul(pr, wq_sb[:, cb],
                                             cr[:, :, dy, :, dx],
                                             start=(mm == 0), stop=(mm == 7),
                                             skip_group_check=True)
                            mm += 1
                cs = slice(icn * 512, (icn + 1) * 512)
                nc.scalar.copy(q4h[0:32, cs], pr)
                for g in range(1, 4):
                    nc.sync.dma_start(q4h[g * 32:(g + 1) * 32, cs],
                                      q4h[0:32, cs])

            def k_chunk(icn):
                ck = pa.tile([128, CB, 16, W], BF16, tag="ck", bufs=2,
                             name="ck")
                nc.gpsimd.dma_start(
                    ck[:],
                    fk_d[:, icn * 16:(icn + 1) * 16, :].rearrange(
                        "(b p) h w -> p b h w", p=128))
                pr = ps.tile([128, 1024], F32, tag="s", bufs=2,
                             name="pr_k")[:M, :512]
                mm = 0
                for cb in range(CB):
                    cr = ck[:, cb].rearrange("c (h dy) (w dx) -> c h dy w dx",
                                             dy=2, dx=2)
                    for dy in range(2):
                        for dx in range(2):
                            nc.tensor.matmul(pr, wk_sb[:, cb],
                                             cr[:, :, dy, :, dx],
                                             start=(mm == 0), stop=(mm == 7),
                                             skip_group_check=True)
                            mm += 1
                for t in range(4):
                    nc.scalar.copy(kh_all[t * 32:(t + 1) * 32, icn, :],
                                   pr[:, t * 128:(t + 1) * 128])

            def v_chunk(icn):
                # load raw chunk into the persistent residual copy
                nc.gpsimd.dma_start(
                    fv_sb[:, :, icn * 16:(icn + 1) * 16, :],
                    fv_d[:, icn * 16:(icn + 1) * 16, :].rearrange(
                        "(b p) h w -> p b h w", p=128))
                # vertical 2x2 pair sums (one DVE pass, bf16 2x mode)
                vv = pa.tile([128, CB, 8, W], BF16, tag="vv", bufs=2,
                             name="vv")
                for cb in range(CB):
                    fvc = fv_sb[:, cb, icn * 16:(icn + 1) * 16, :].rearrange(
                        "c (h dy) w -> c h dy w", dy=2)
                    nc.vector.tensor_add(vv[:, cb], fvc[:, :, 0], fvc[:, :, 1])
                # vT projection per j-block; horizontal fold via 2 strided
                # lhsT terms per cb.
                for r in range(4):
                    jb = icn * 4 + r
                    vt_ps = ps.tile([128, 1024], F32, tag="s", bufs=2,
                                    name="vt_ps")[:, :C]
                    mm = 0
                    for cb in range(CB):
                        vr = vv[:, cb, 2 * r:2 * r + 2, :].rearrange(
                            "c h (w dx) -> c h w dx", dx=2)
                        for dx in range(2):
                            nc.tensor.matmul(vt_ps, vr[:, :, :, dx],
                                             wv_sb[:, cb],
                                             start=(mm == 0), stop=(mm == 3),
                                             skip_group_check=True)
                            mm += 1
                    nc.scalar.copy(vt_all[:, jb, :], vt_ps)

            def b_s_exp(ic, jg, t1s, t2s):
                i0 = ic * 512
                # allocate both S tiles, then issue all 4 matmuls adjacent so
                # the four 32-row tile_position groups run concurrently.
                s_t = [ps.tile([128, 1024], F32, tag="s", bufs=2, name="s_u")
                       for _ in range(2)]
                for t in range(4):
                    gs = slice(t * 32, (t + 1) * 32)
                    nc.tensor.matmul(
                        s_t[t // 2][:, (t % 2) * 512:(t % 2) * 512 + 512],
                        kh_all[gs, jg, :], q4h[gs, i0:i0 + 512],
                        start=True, stop=True,
                        tile_position=(t * 32, 0),
                        skip_group_check=True)
                p_t = []
                for u in range(2):
                    p = pa.tile([128, 1024], BF16, tag="p", bufs=16, name="p")
                    nc.scalar.activation(p[:], s_t[u][:], AF.Exp, scale=0.0625)
                    p_t.append(p)
                # denominator tree level 1 (jg 1,3 -> GPSIMD for balance),
                # level 2 inlined on odd jg, first half of level 3 after jg 3.
                t1 = pa.tile([128, 1024], BF16, tag="t1", bufs=3, name="t1")
                if ic > 1 and jg in (1, 3):
                    nc.gpsimd.tensor_add(t1[:], p_t[0][:], p_t[1][:])
                else:
                    nc.vector.tensor_add(t1[:], p_t[0][:], p_t[1][:])
                t1s.append(t1)
                if jg % 2 == 1:
                    t2 = pa.tile([128, 1024], BF16, tag="t2", bufs=4,
                                 name="t2")
                    nc.vector.tensor_add(t2[:], t1s[-2][:], t1s[-1][:])
                    t2s.append(t2)
                if jg == 3:
                    t3 = pa.tile([128, 1024], BF16, tag="t3", bufs=2,
                                 name="t3")
                    nc.vector.tensor_add(t3[:], t2s[0][:], t2s[1][:])
                    t2s.append(t3)  # t2s = [t2_0, t2_1, t3a, t2_2, t2_3]
                if jg == JG - 1:
                    t3b = pa.tile([128, 1024], BF16, tag="t3", bufs=2,
                                  name="t3b")
                    nc.vector.tensor_add(t3b[:], t2s[3][:], t2s[4][:])
                    tT = pa.tile([128, 1024], BF16, tag="tT", bufs=2,
                                 name="tT")
                    nc.vector.tensor_add(tT[:], t2s[2][:], t3b[:])
                    t2s.append(tT)  # t2s[5] = tT
                return p_t

            def b_pv(jg, o_ps, p_t):
                for u in range(2):
                    for tt in range(2):
                        j = jg * 4 + u * 2 + tt
                        pr = p_t[u][:, tt * 512:tt * 512 + 512]
                        for cb in range(CB):
                            nc.tensor.matmul(
                                o_ps[cb],
                                vt_all[:, j, cb * 128:(cb + 1) * 128],
                                pr,
                                start=(j == 0), stop=(j == JB - 1),
                                skip_group_check=True)

            def b_tail1(ic, o_ps, t1s, t2s):
                tT = t2s[5]  # tree finished inline at jg==7
                # l = column sums of both halves via 2 ones-matmuls
                l_ps = ps.tile([128, 1024], F32, tag="s", bufs=2,
                               name="l_ps")[:1, :512]
                for u in range(2):
                    nc.tensor.matmul(l_ps, ones_b[:],
                                     tT[:, u * 512:(u + 1) * 512],
                                     start=(u == 0), stop=(u == 1),
                                     skip_group_check=True)
                l_sb = pa.tile([1, 512], F32, tag="l_sb", bufs=2, name="l_sb")
                nc.scalar.copy(l_sb[:], l_ps)
                # transpose to (128, 4) via DRAM bounce, reciprocal, back
                l_dr = dpool.tile([512], F32, tag="l_dr", bufs=2, name="l_dr")
                nc.sync.dma_start(l_dr[:], l_sb[:])
                lT = pa.tile([128, 4], F32, tag="lT", bufs=2, name="lT")
                nc.sync.dma_start(lT[:], l_dr.rearrange("(p b) -> p b", b=4))
                rT = pa.tile([128, 4], F32, tag="rT", bufs=2, name="rT")
                nc.vector.reciprocal(rT[:], lT[:])
                r_dr = dpool.tile([512], F32, tag="r_dr", bufs=2, name="r_dr")
                nc.sync.dma_start(r_dr.rearrange("(p b) -> p b", b=4), rT[:])
                rb_sb = pa.tile([128, 512], F32, tag="rb_sb", bufs=2,
                                name="rb_sb")
                nc.sync.dma_start(
                    rb_sb[:],
                    r_dr.rearrange("(o x) -> o x", o=1).to_broadcast(
                        (128, 512)))
                return o_ps, rb_sb

            def b_tail2(ic, o_ps, rb_sb):
                for cb in range(CB):
                    oc = pa.tile([128, 512], F32, tag="oc", bufs=2, name="oc")
                    nc.vector.tensor_mul(oc[:], o_ps[cb][:], rb_sb[:])
                    for sl in range(2):
                        final = pa.tile([128, 4, 2, WP, 2], F32, tag="final",
                                        bufs=3, name="final")
                        up = oc.rearrange("c (h w) -> c h w", w=WP)[
                            :, sl * 4:(sl + 1) * 4, :, None].to_broadcast(
                                (128, 4, WP, 2))
                        r0 = ic * 16 + sl * 8
                        fvv = fv_sb[:, cb, r0:r0 + 8, :].rearrange(
                            "c (h dy) (w dx) -> c h dy w dx", dy=2, dx=2)
                        nc.vector.tensor_add(final[:, :, 0], up, fvv[:, :, 0])
                        nc.gpsimd.tensor_add(final[:, :, 1], up,
                                             fvv[:, :, 1])
                        nc.sync.dma_start(
                            out_d[cb * 128:(cb + 1) * 128, r0:r0 + 8, :],
                            final.rearrange("c h dy w dx -> c (h dy) (w dx)"))

            # ================= schedule =================
            # i-chunk 0's S+exp chases the fk stream; i-chunks 0 and 1 both
            # chase the fv stream (PV for ic0, full pipeline for ic1); all
            # q projections are precomputed during the streams. i-chunks
            # 2..7 run as one flat software-pipelined (ic, jg) loop with
            # the S+exp stage one unit ahead and epilogue-applies deferred
            # into the following i-chunk.
            st = {}

            def new_state(ic):
                st[ic] = dict(
                    o=[ps.tile([128, 512], F32, tag=f"o{cb}", bufs=2,
                               name=f"o{cb}_ps") for cb in range(CB)],
                    t1s=[], t2s=[], p={})

            def s_exp_unit(ic, jg):
                if jg == 0:
                    new_state(ic)
                s = st[ic]
                s["p"][jg] = b_s_exp(ic, jg, s["t1s"], s["t2s"])

            pending = []

            q_chunk(0)
            for icn in range(IC):
                k_chunk(icn)
                s_exp_unit(0, icn)
            q_chunk(1)
            for icn in range(IC):
                v_chunk(icn)
                if icn >= 2 and icn <= 7:
                    q_chunk(icn)  # prefetch remaining q chunks
                b_pv(icn, st[0]["o"], st[0]["p"].pop(icn))
                s_exp_unit(1, icn)
                b_pv(icn, st[1]["o"], st[1]["p"].pop(icn))
            pending.append((0,) + b_tail1(0, st[0]["o"], st[0]["t1s"],
                                          st[0]["t2s"]))
            del st[0]
            pending.append((1,) + b_tail1(1, st[1]["o"], st[1]["t1s"],
                                          st[1]["t2s"]))
            del st[1]

            units = [(ic, jg) for ic in range(2, IC) for jg in range(JG)]
            s_exp_unit(2, 0)
            for idx, (ic, jg) in enumerate(units):
                if idx + 1 < len(units):
                    s_exp_unit(*units[idx + 1])
                b_pv(jg, st[ic]["o"], st[ic]["p"].pop(jg))
                if jg in (2, 5) and pending:
                    b_tail2(*pending.pop(0))
                if jg == JG - 1:
                    pending.append(
                        (ic,) + b_tail1(ic, st[ic]["o"], st[ic]["t1s"],
                                        st[ic]["t2s"]))
                    del st[ic]
            while pending:
                b_tail2(*pending.pop(0))

    nc.compile()
    return nc


_NC_CACHE = []
LAST_RESULT = []  # last BassKernelResults, for perf inspection by test.py


def kernel(**inputs) -> np.ndarray:
    fq = np.ascontiguousarray(np.asarray(inputs["feature_q"], dtype=np.float32))
    fk = np.ascontiguousarray(np.asarray(inputs["feature_k"], dtype=np.float32))
    fv = np.ascontiguousarray(np.asarray(inputs["feature_v"], dtype=np.float32))
    wq = np.asarray(inputs["Wq"], dtype=np.float32)
    wk = np.asarray(inputs["Wk"], dtype=np.float32)
    wv = np.asarray(inputs["Wv"], dtype=np.float32)

    # weight layout prep (pure layout/scale folding, no heavy compute):
    # on-device pooling is a 2x2 *sum*; q,k each pick up 4x -> s is 16x,
    # folded into the on-device exp scale; v's 4x is folded into WvT here.
    wqt = np.ascontiguousarray(wq.T.astype(ml_dtypes.bfloat16))
    wkt = np.ascontiguousarray(wk.T.astype(ml_dtypes.bfloat16))
    wvt = np.ascontiguousarray(
        (wv.T * 0.25).astype(ml_dtypes.bfloat16))     # (C, C) [c_in, c_out]

    if not _NC_CACHE:
        _NC_CACHE.append(build_module())
    nc = _NC_CACHE[0]

    in_maps = [
        {
            "feature_q": fq[b],
            "feature_k": fk[b],
            "feature_v": fv[b],
            "WqT": wqt,
            "WkT": wkt,
            "WvT": wvt,
        }
        for b in range(B)
    ]
    res = run_bass_kernel_spmd(nc, in_maps, core_ids=list(range(B)))
    LAST_RESULT.clear()
    LAST_RESULT.append(res)
    out = np.stack([res.results[b]["out"] for b in range(B)], axis=0)
    return out.astype(np.float32)


if __name__ == "__main__":
    nc = build_module()
    print("module built + compiled OK")
